# revision 16
# baseline (speedup 1.0000x reference)
"""Multi-head attention (B=8, N=2048, C=320, H=5, D=64) on 8 Trainium2 cores.

Sharding: data-parallel over batch — core b computes attention for x[b].
Weights are replicated. No collectives.

Per-core strategy (fp16 operands, fp32 accumulation):
  - x^T [C, N] via PE transpose (c on partitions) feeds both qkv matmuls.
  - q^T, k^T computed transposed [C_qk, N]; v computed natural [N, H*(D+1)]
    with a ones column appended per head.
  - Scores computed transposed: S^T[m, n] = sum_d k^T[d,m] q^T[d,n], so the
    AV matmul can consume P^T = exp(S^T) directly as its moving operand with
    lhsT = [V_h | ones]; row 64 of the AV output is the softmax denominator.
  - Normalization is deferred: per-head projection PSUM tiles are combined
    with scalar_tensor_tensor((y_h * recip_h) + acc), bias as initial acc.
  - qkv planes are interleaved with attention head-pairs so the scalar
    engine (exp) starts early instead of idling through the whole qkv phase.
"""

import numpy as np

import concourse.bacc as bacc
import concourse.tile as tile
from concourse import mybir
from concourse.bass_utils import run_bass_kernel_spmd
from concourse.masks import make_identity

FP32 = mybir.dt.float32
FP16 = mybir.dt.float16
AF = mybir.ActivationFunctionType
ALU = mybir.AluOpType

B = 8
C = 320
H = 5
D = 64
SCALE = D ** -0.5
# c-dim tiles of 128/128/64 (contraction tiles for the C=320 dim)
CT = [(0, 128), (128, 128), (256, 64)]


def build_program(N: int):
    """Build + compile the single-core Bass program (SPMD across 8 cores)."""
    nc = bacc.Bacc("TRN2", target_bir_lowering=False, debug=False)

    x_d = nc.dram_tensor("x", [N, C], FP32, kind="ExternalInput")
    wqkv_d = nc.dram_tensor("w_qkv", [3 * C, C], FP32, kind="ExternalInput")
    wproj_d = nc.dram_tensor("w_proj", [C, C], FP32, kind="ExternalInput")
    bproj_d = nc.dram_tensor("b_proj", [C], FP32, kind="ExternalInput")
    out_d = nc.dram_tensor("out", [N, C], FP32, kind="ExternalOutput")

    MT = N // 128                       # number of 128-row seq tiles
    CHUNK = 1024 if N % 1024 == 0 else N
    NCH = N // CHUNK                    # attention n-chunks
    PT_CH = CHUNK // 128                # proj n-tiles per chunk

    with tile.TileContext(nc) as tc:
        with (
            tc.tile_pool(name="persist", bufs=1) as per,
            tc.tile_pool(name="ld", bufs=3) as ld,
            tc.tile_pool(name="sc_ps", bufs=2, space="PSUM") as sc_ps,
            tc.tile_pool(name="s_ps", bufs=2, space="PSUM") as s_ps,
            tc.tile_pool(name="o_ps", bufs=1, space="PSUM") as o_ps,
            tc.tile_pool(name="pt", bufs=4) as pt_pool,
            tc.tile_pool(name="yacc", bufs=4) as yacc_pool,
        ):
            identity = per.tile([128, 128], FP32)
            make_identity(nc, identity[:])
            identity_h = per.tile([128, 128], FP16)
            nc.vector.tensor_copy(identity_h[:], identity[:])

            wpt = per.tile([64, H, C], FP16)    # w_proj^T per head, offset 0
            xT = per.tile([128, 3, N], FP16)
            wT = per.tile([128, 3, 3 * C], FP16)
            qT = per.tile([128, 3, N], FP16)    # plane j: w_qkv rows j*128..
            kT = per.tile([128, 3, N], FP16)
            v_sb = per.tile([128, MT, H * (D + 1)], FP16)
            oT = per.tile([65, H, N], FP16)     # rows 0-63: O^T_h
            denom = per.tile([H, N], FP32)
            recipT = per.tile([128, MT, H], FP32)
            bias_sb = per.tile([128, C], FP32)
            b_row = per.tile([1, C], FP32)
            ones1 = per.tile([1, 128], FP32)
            nc.gpsimd.memset(ones1[:], 1.0)

            # ones column per head in v (written once; evictions skip it)
            v_heads = v_sb[:].rearrange("p m (h e) -> p m h e", h=H)
            nc.gpsimd.memset(v_heads[:, :, :, D : D + 1], 1.0)

            def transpose_fp16(dst_ap, src_ap, rp, cp):
                """dst[cp, rp] = src[rp, cp].T — DMA xbar when the block shape
                allows (keeps PE/DVE free during startup), else PE transpose."""
                if cp % 128 == 0 and rp % 16 == 0:
                    nc.sync.dma_start_transpose(dst_ap, src_ap)
                    return
                ps = sc_ps.tile([128, 512], FP16, tag="sc")
                nc.tensor.transpose(ps[:cp, :rp], src_ap, identity_h[:rp, :rp])
                nc.vector.tensor_copy(dst_ap, ps[:cp, :rp])

            # ---- w_qkv -> wT (w_qkv^T) ----
            for wt in range((3 * C + 127) // 128):
                r0 = wt * 128
                rp = min(128, 3 * C - r0)
                wnat = ld.tile([128, C], FP32, tag="wnat")
                nc.sync.dma_start(wnat[:rp, :], wqkv_d.ap()[r0 : r0 + rp, :])
                wnat_h = ld.tile([128, C], FP16, tag="wnat_h")
                nc.vector.tensor_copy(wnat_h[:rp, :], wnat[:rp, :])
                for ci, (c0, cp) in enumerate(CT):
                    transpose_fp16(
                        wT[:cp, ci, r0 : r0 + rp],
                        wnat_h[:rp, c0 : c0 + cp],
                        rp,
                        cp,
                    )

            # ---- w_proj -> wpt (w_proj^T, per-head planes) ----
            for wt, (r0, rp) in enumerate(CT):
                wpnat = ld.tile([128, C], FP32, tag="wnat")
                nc.sync.dma_start(wpnat[:rp, :], wproj_d.ap()[r0 : r0 + rp, :])
                wpnat_h = ld.tile([128, C], FP16, tag="wnat_h")
                nc.vector.tensor_copy(wpnat_h[:rp, :], wpnat[:rp, :])
                for h in range(H):
                    transpose_fp16(
                        wpt[:, h, r0 : r0 + rp],
                        wpnat_h[:rp, h * D : (h + 1) * D],
                        rp,
                        D,
                    )

            # ---- bias broadcast [128, C] ----
            nc.sync.dma_start(b_row[:], bproj_d.ap().rearrange("(a c) -> a c", a=1))
            ps = sc_ps.tile([128, 512], FP32, tag="sc")
            nc.tensor.matmul(ps[:, :C], ones1[:], b_row[:], start=True, stop=True)
            nc.vector.tensor_copy(bias_sb[:], ps[:, :C])

            # ---- x -> xT ----
            x_re = x_d.ap().rearrange("(t p) c -> p t c", p=128)
            for g in range(0, MT, 4):
                gn = min(4, MT - g)
                xnat = ld.tile([128, 4, C], FP32, tag="xnat")
                nc.sync.dma_start(xnat[:, :gn, :], x_re[:, g : g + gn, :])
                xnat_h = ld.tile([128, 4, C], FP16, tag="xnat_h")
                nc.vector.tensor_copy(xnat_h[:, :gn, :], xnat[:, :gn, :])
                for t in range(gn):
                    mt = g + t
                    for ci, (c0, cp) in enumerate(CT):
                        transpose_fp16(
                            xT[:cp, ci, mt * 128 : (mt + 1) * 128],
                            xnat_h[:, t, c0 : c0 + cp],
                            128,
                            cp,
                        )

            # ---- v natural: lhsT = xT slice, rhs = wT v-columns ----
            for mt in range(MT):
                ps = sc_ps.tile([128, 512], FP32, tag="sc")
                for ci, (c0, cp) in enumerate(CT):
                    nc.tensor.matmul(
                        ps[:, :C],
                        xT[:cp, ci, mt * 128 : (mt + 1) * 128],
                        wT[:cp, ci, 2 * C : 3 * C],
                        start=(ci == 0),
                        stop=(ci == 2),
                    )
                nc.vector.tensor_copy(
                    v_heads[:, mt, :, 0:D],
                    ps[:, :C].rearrange("p (h e) -> p h e", h=H),
                )

            def emit_qk_plane(j):
                r0, rp = CT[j]
                for dst, base in ((qT, 0), (kT, C)):
                    for s0 in range(0, N, 512):
                        sw = min(512, N - s0)
                        ps = sc_ps.tile([128, 512], FP32, tag="sc")
                        for ci, (c0, cp) in enumerate(CT):
                            nc.tensor.matmul(
                                ps[:rp, :sw],
                                wT[:cp, ci, base + r0 : base + r0 + rp],
                                xT[:cp, ci, s0 : s0 + sw],
                                start=(ci == 0),
                                stop=(ci == 2),
                            )
                        nc.vector.tensor_copy(dst[:rp, j, s0 : s0 + sw], ps[:rp, :sw])

            def emit_attention(h, nci, interleave=()):
                jobs = list(interleave)
                n0 = nci * CHUNK
                jt = h // 2
                off = 64 * (h % 2)
                ot_ps = o_ps.tile([65, CHUNK], FP32, tag="ot")
                for mt in range(MT):
                    if jobs:
                        jobs.pop(0)()
                    sp = s_ps.tile([128, CHUNK], FP32, tag="s")
                    for s0 in range(0, CHUNK, 512):
                        sw = min(512, CHUNK - s0)
                        nc.tensor.matmul(
                            sp[:, s0 : s0 + sw],
                            kT[off : off + D, jt, mt * 128 : (mt + 1) * 128],
                            qT[off : off + D, jt, n0 + s0 : n0 + s0 + sw],
                            start=True,
                            stop=True,
                        )
                    pt = pt_pool.tile([128, CHUNK], FP16, tag="pt")
                    nc.scalar.activation(pt[:], sp[:], AF.Exp, scale=SCALE)
                    for s0 in range(0, CHUNK, 512):
                        sw = min(512, CHUNK - s0)
                        nc.tensor.matmul(
                            ot_ps[:, s0 : s0 + sw],
                            v_sb[:, mt, h * (D + 1) : (h + 1) * (D + 1)],
                            pt[:, s0 : s0 + sw],
                            start=(mt == 0),
                            stop=(mt == MT - 1),
                        )
                nc.vector.tensor_copy(oT[0:64, h, n0 : n0 + CHUNK], ot_ps[0:64, :])
                dstage = yacc_pool.tile([65, CHUNK], FP32, tag="dst")
                nc.vector.tensor_copy(dstage[64:65, :], ot_ps[64:65, :])
                nc.sync.dma_start(denom[h : h + 1, n0 : n0 + CHUNK], dstage[64:65, :])
                for job in jobs:
                    job()

            def emit_proj_prep(nci):
                # denominators -> transposed layout -> one cheap reciprocal
                for t in range(PT_CH):
                    gt = nci * PT_CH + t
                    ps = sc_ps.tile([128, 512], FP32, tag="sc")
                    nc.tensor.transpose(
                        ps[:, :H],
                        denom[:, gt * 128 : (gt + 1) * 128],
                        identity[:H, :H],
                    )
                    nc.vector.tensor_copy(recipT[:, gt, :], ps[:, :H])
                rt = recipT[:, nci * PT_CH : (nci + 1) * PT_CH, :]
                nc.vector.reciprocal(rt, rt)

            def emit_proj_tile(gt):
                acc = None
                for h in range(H):
                    yp = sc_ps.tile([128, 512], FP32, tag="sc")
                    nc.tensor.matmul(
                        yp[:, :C],
                        oT[0:64, h, gt * 128 : (gt + 1) * 128],
                        wpt[:, h, :],
                        start=True,
                        stop=True,
                    )
                    nacc = yacc_pool.tile([128, C], FP32, tag="acc")
                    prev = bias_sb if acc is None else acc
                    nc.vector.scalar_tensor_tensor(
                        nacc[:],
                        yp[:, :C],
                        recipT[:, gt, h : h + 1],
                        prev[:],
                        ALU.mult,
                        ALU.add,
                    )
                    acc = nacc
                nc.sync.dma_start(out_d.ap()[gt * 128 : (gt + 1) * 128, :], acc[:])

            def proj_jobs(nci):
                jobs = [lambda n=nci: emit_proj_prep(n)]
                for t in range(PT_CH):
                    jobs.append(lambda g=nci * PT_CH + t: emit_proj_tile(g))
                return jobs

            # interleave qkv with attention so ACT starts early
            emit_qk_plane(0)
            for nci in range(NCH):
                emit_attention(0, nci)
                emit_attention(1, nci)
            emit_qk_plane(1)
            for nci in range(NCH):
                emit_attention(2, nci)
                emit_attention(3, nci)
            emit_qk_plane(2)
            pending = ()
            for nci in range(NCH):
                emit_attention(4, nci, interleave=pending)
                pending = proj_jobs(nci)
            for job in pending:
                job()

    nc.compile()
    return nc


_cache = {}


def _get_program(N: int):
    if N not in _cache:
        _cache[N] = build_program(N)
    return _cache[N]


def kernel(x, w_qkv, w_proj, b_proj):
    x = np.ascontiguousarray(np.asarray(x, dtype=np.float32))
    w_qkv = np.ascontiguousarray(np.asarray(w_qkv, dtype=np.float32))
    w_proj = np.ascontiguousarray(np.asarray(w_proj, dtype=np.float32))
    b_proj = np.ascontiguousarray(np.asarray(b_proj, dtype=np.float32))
    Bx, N, Cx = x.shape
    assert Bx == B and Cx == C, (x.shape,)

    nc = _get_program(N)
    in_maps = [
        {"x": x[b], "w_qkv": w_qkv, "w_proj": w_proj, "b_proj": b_proj}
        for b in range(B)
    ]
    res = run_bass_kernel_spmd(nc, in_maps, core_ids=list(range(B)))
    return np.stack([res.results[b]["out"] for b in range(B)], axis=0)


# revision 17
# speedup vs baseline: 1.1329x; 1.1329x over previous
"""Multi-head attention (B=8, N=2048, C=320, H=5, D=64) on 8 Trainium2 cores.

Sharding: data-parallel over batch — core b computes attention for x[b].
Weights are replicated. No collectives.

Per-core strategy (fp16 operands, fp32 accumulation):
  - x^T [C, N] via PE transpose (c on partitions) feeds both qkv matmuls.
  - q^T, k^T computed transposed [C_qk, N]; v computed natural [N, H*(D+1)]
    with a ones column appended per head.
  - Scores computed transposed: S^T[m, n] = sum_d k^T[d,m] q^T[d,n], so the
    AV matmul can consume P^T = exp(S^T) directly as its moving operand with
    lhsT = [V_h | ones]; row 64 of the AV output is the softmax denominator.
  - Normalization is deferred: per-head projection PSUM tiles are combined
    with scalar_tensor_tensor((y_h * recip_h) + acc), bias as initial acc.
  - qkv planes are interleaved with attention head-pairs so the scalar
    engine (exp) starts early instead of idling through the whole qkv phase.
"""

import numpy as np

import concourse.bacc as bacc
import concourse.tile as tile
from concourse import mybir
from concourse.bass_utils import run_bass_kernel_spmd
from concourse.masks import make_identity

FP32 = mybir.dt.float32
FP16 = mybir.dt.float16
AF = mybir.ActivationFunctionType
ALU = mybir.AluOpType

B = 8
C = 320
H = 5
D = 64
SCALE = D ** -0.5
# c-dim tiles of 128/128/64 (contraction tiles for the C=320 dim)
CT = [(0, 128), (128, 128), (256, 64)]


def build_program(N: int):
    """Build + compile the single-core Bass program (SPMD across 8 cores)."""
    nc = bacc.Bacc("TRN2", target_bir_lowering=False, debug=False)

    x_d = nc.dram_tensor("x", [N, C], FP32, kind="ExternalInput")
    wqkv_d = nc.dram_tensor("w_qkv", [3 * C, C], FP32, kind="ExternalInput")
    wproj_d = nc.dram_tensor("w_proj", [C, C], FP32, kind="ExternalInput")
    bproj_d = nc.dram_tensor("b_proj", [C], FP32, kind="ExternalInput")
    out_d = nc.dram_tensor("out", [N, C], FP32, kind="ExternalOutput")

    MT = N // 128                       # number of 128-row seq tiles
    CHUNK = 1024 if N % 1024 == 0 else N
    NCH = N // CHUNK                    # attention n-chunks
    PT_CH = CHUNK // 128                # proj n-tiles per chunk

    with tile.TileContext(nc) as tc:
        with (
            tc.tile_pool(name="persist", bufs=1) as per,
            tc.tile_pool(name="ld", bufs=3) as ld,
            tc.tile_pool(name="sc_ps", bufs=2, space="PSUM") as sc_ps,
            tc.tile_pool(name="s_ps", bufs=2, space="PSUM") as s_ps,
            tc.tile_pool(name="o_ps", bufs=1, space="PSUM") as o_ps,
            tc.tile_pool(name="pt", bufs=4) as pt_pool,
            tc.tile_pool(name="yacc", bufs=4) as yacc_pool,
        ):
            identity = per.tile([128, 128], FP32)
            make_identity(nc, identity[:])
            identity_h = per.tile([128, 128], FP16)
            nc.vector.tensor_copy(identity_h[:], identity[:])

            wpt = per.tile([64, H, C], FP16)    # w_proj^T per head, offset 0
            xT = per.tile([128, 3, N], FP16)
            wT = per.tile([128, 3, 3 * C], FP16)
            qT = per.tile([128, 3, N], FP16)    # plane j: w_qkv rows j*128..
            kT = per.tile([128, 3, N], FP16)
            v_sb = per.tile([128, MT, H * (D + 1)], FP16)
            oT = per.tile([65, H, N], FP16)     # rows 0-63: O^T_h
            denom = per.tile([H, N], FP32)
            recipT = per.tile([128, MT, H], FP32)
            bias_sb = per.tile([128, C], FP32)
            b_row = per.tile([1, C], FP32)
            ones1 = per.tile([1, 128], FP32)
            nc.gpsimd.memset(ones1[:], 1.0)

            # ones column per head in v (written once; evictions skip it)
            v_heads = v_sb[:].rearrange("p m (h e) -> p m h e", h=H)
            nc.gpsimd.memset(v_heads[:, :, :, D : D + 1], 1.0)

            def transpose_fp16(dst_ap, src_ap, rp, cp):
                """dst[cp, rp] = src[rp, cp].T — DMA xbar when the block shape
                allows (keeps PE/DVE free during startup), else PE transpose."""
                ps = sc_ps.tile([128, 512], FP16, tag="sc")
                nc.tensor.transpose(ps[:cp, :rp], src_ap, identity_h[:rp, :rp])
                nc.vector.tensor_copy(dst_ap, ps[:cp, :rp])

            # ---- w_qkv -> wT (w_qkv^T) ----
            for wt in range((3 * C + 127) // 128):
                r0 = wt * 128
                rp = min(128, 3 * C - r0)
                wnat = ld.tile([128, C], FP32, tag="wnat")
                nc.sync.dma_start(wnat[:rp, :], wqkv_d.ap()[r0 : r0 + rp, :])
                wnat_h = ld.tile([128, C], FP16, tag="wnat_h")
                nc.vector.tensor_copy(wnat_h[:rp, :], wnat[:rp, :])
                for ci, (c0, cp) in enumerate(CT):
                    transpose_fp16(
                        wT[:cp, ci, r0 : r0 + rp],
                        wnat_h[:rp, c0 : c0 + cp],
                        rp,
                        cp,
                    )

            # ---- w_proj -> wpt (w_proj^T, per-head planes) ----
            for wt, (r0, rp) in enumerate(CT):
                wpnat = ld.tile([128, C], FP32, tag="wnat")
                nc.sync.dma_start(wpnat[:rp, :], wproj_d.ap()[r0 : r0 + rp, :])
                wpnat_h = ld.tile([128, C], FP16, tag="wnat_h")
                nc.vector.tensor_copy(wpnat_h[:rp, :], wpnat[:rp, :])
                for h in range(H):
                    transpose_fp16(
                        wpt[:, h, r0 : r0 + rp],
                        wpnat_h[:rp, h * D : (h + 1) * D],
                        rp,
                        D,
                    )

            # ---- bias broadcast [128, C] ----
            nc.sync.dma_start(b_row[:], bproj_d.ap().rearrange("(a c) -> a c", a=1))
            ps = sc_ps.tile([128, 512], FP32, tag="sc")
            nc.tensor.matmul(ps[:, :C], ones1[:], b_row[:], start=True, stop=True)
            nc.vector.tensor_copy(bias_sb[:], ps[:, :C])

            # ---- x -> xT ----
            x_re = x_d.ap().rearrange("(t p) c -> p t c", p=128)
            for g in range(0, MT, 4):
                gn = min(4, MT - g)
                xnat = ld.tile([128, 4, C], FP32, tag="xnat")
                nc.sync.dma_start(xnat[:, :gn, :], x_re[:, g : g + gn, :])
                xnat_h = ld.tile([128, 4, C], FP16, tag="xnat_h")
                nc.vector.tensor_copy(xnat_h[:, :gn, :], xnat[:, :gn, :])
                for t in range(gn):
                    mt = g + t
                    for ci, (c0, cp) in enumerate(CT):
                        transpose_fp16(
                            xT[:cp, ci, mt * 128 : (mt + 1) * 128],
                            xnat_h[:, t, c0 : c0 + cp],
                            128,
                            cp,
                        )

            # ---- v natural: lhsT = xT slice, rhs = wT v-columns ----
            for mt in range(MT):
                ps = sc_ps.tile([128, 512], FP32, tag="sc")
                for ci, (c0, cp) in enumerate(CT):
                    nc.tensor.matmul(
                        ps[:, :C],
                        xT[:cp, ci, mt * 128 : (mt + 1) * 128],
                        wT[:cp, ci, 2 * C : 3 * C],
                        start=(ci == 0),
                        stop=(ci == 2),
                    )
                nc.vector.tensor_copy(
                    v_heads[:, mt, :, 0:D],
                    ps[:, :C].rearrange("p (h e) -> p h e", h=H),
                )

            def emit_qk_plane(j):
                r0, rp = CT[j]
                for dst, base in ((qT, 0), (kT, C)):
                    for s0 in range(0, N, 512):
                        sw = min(512, N - s0)
                        ps = sc_ps.tile([128, 512], FP32, tag="sc")
                        for ci, (c0, cp) in enumerate(CT):
                            nc.tensor.matmul(
                                ps[:rp, :sw],
                                wT[:cp, ci, base + r0 : base + r0 + rp],
                                xT[:cp, ci, s0 : s0 + sw],
                                start=(ci == 0),
                                stop=(ci == 2),
                            )
                        nc.vector.tensor_copy(dst[:rp, j, s0 : s0 + sw], ps[:rp, :sw])

            def emit_attention(h, nci, interleave=()):
                jobs = list(interleave)
                n0 = nci * CHUNK
                jt = h // 2
                off = 64 * (h % 2)
                ot_ps = o_ps.tile([65, CHUNK], FP32, tag="ot")
                for mt in range(MT):
                    if jobs:
                        jobs.pop(0)()
                    sp = s_ps.tile([128, CHUNK], FP32, tag="s")
                    for s0 in range(0, CHUNK, 512):
                        sw = min(512, CHUNK - s0)
                        nc.tensor.matmul(
                            sp[:, s0 : s0 + sw],
                            kT[off : off + D, jt, mt * 128 : (mt + 1) * 128],
                            qT[off : off + D, jt, n0 + s0 : n0 + s0 + sw],
                            start=True,
                            stop=True,
                        )
                    pt = pt_pool.tile([128, CHUNK], FP16, tag="pt")
                    nc.scalar.activation(pt[:], sp[:], AF.Exp, scale=SCALE)
                    for s0 in range(0, CHUNK, 512):
                        sw = min(512, CHUNK - s0)
                        nc.tensor.matmul(
                            ot_ps[:, s0 : s0 + sw],
                            v_sb[:, mt, h * (D + 1) : (h + 1) * (D + 1)],
                            pt[:, s0 : s0 + sw],
                            start=(mt == 0),
                            stop=(mt == MT - 1),
                        )
                nc.vector.tensor_copy(oT[0:64, h, n0 : n0 + CHUNK], ot_ps[0:64, :])
                dstage = yacc_pool.tile([65, CHUNK], FP32, tag="dst")
                nc.vector.tensor_copy(dstage[64:65, :], ot_ps[64:65, :])
                nc.sync.dma_start(denom[h : h + 1, n0 : n0 + CHUNK], dstage[64:65, :])
                for job in jobs:
                    job()

            def emit_proj_prep(nci):
                # denominators -> transposed layout -> one cheap reciprocal
                for t in range(PT_CH):
                    gt = nci * PT_CH + t
                    ps = sc_ps.tile([128, 512], FP32, tag="sc")
                    nc.tensor.transpose(
                        ps[:, :H],
                        denom[:, gt * 128 : (gt + 1) * 128],
                        identity[:H, :H],
                    )
                    nc.vector.tensor_copy(recipT[:, gt, :], ps[:, :H])
                rt = recipT[:, nci * PT_CH : (nci + 1) * PT_CH, :]
                nc.vector.reciprocal(rt, rt)

            def emit_proj_tile(gt):
                acc = None
                for h in range(H):
                    yp = sc_ps.tile([128, 512], FP32, tag="sc")
                    nc.tensor.matmul(
                        yp[:, :C],
                        oT[0:64, h, gt * 128 : (gt + 1) * 128],
                        wpt[:, h, :],
                        start=True,
                        stop=True,
                    )
                    nacc = yacc_pool.tile([128, C], FP32, tag="acc")
                    prev = bias_sb if acc is None else acc
                    nc.vector.scalar_tensor_tensor(
                        nacc[:],
                        yp[:, :C],
                        recipT[:, gt, h : h + 1],
                        prev[:],
                        ALU.mult,
                        ALU.add,
                    )
                    acc = nacc
                nc.sync.dma_start(out_d.ap()[gt * 128 : (gt + 1) * 128, :], acc[:])

            def proj_jobs(nci):
                jobs = [lambda n=nci: emit_proj_prep(n)]
                for t in range(PT_CH):
                    jobs.append(lambda g=nci * PT_CH + t: emit_proj_tile(g))
                return jobs

            # interleave qkv with attention so ACT starts early
            emit_qk_plane(0)
            for nci in range(NCH):
                emit_attention(0, nci)
                emit_attention(1, nci)
            emit_qk_plane(1)
            for nci in range(NCH):
                emit_attention(2, nci)
                emit_attention(3, nci)
            emit_qk_plane(2)
            pending = ()
            for nci in range(NCH):
                emit_attention(4, nci, interleave=pending)
                pending = proj_jobs(nci)
            for job in pending:
                job()

    nc.compile()
    return nc


_cache = {}


def _get_program(N: int):
    if N not in _cache:
        _cache[N] = build_program(N)
    return _cache[N]


def kernel(x, w_qkv, w_proj, b_proj):
    x = np.ascontiguousarray(np.asarray(x, dtype=np.float32))
    w_qkv = np.ascontiguousarray(np.asarray(w_qkv, dtype=np.float32))
    w_proj = np.ascontiguousarray(np.asarray(w_proj, dtype=np.float32))
    b_proj = np.ascontiguousarray(np.asarray(b_proj, dtype=np.float32))
    Bx, N, Cx = x.shape
    assert Bx == B and Cx == C, (x.shape,)

    nc = _get_program(N)
    in_maps = [
        {"x": x[b], "w_qkv": w_qkv, "w_proj": w_proj, "b_proj": b_proj}
        for b in range(B)
    ]
    res = run_bass_kernel_spmd(nc, in_maps, core_ids=list(range(B)))
    return np.stack([res.results[b]["out"] for b in range(B)], axis=0)
